# revision 13
# baseline (speedup 1.0000x reference)
"""Trainium2 Bass kernel for nn_DenseAttnProcessor (sparse_attention).

Cross-attention block: q = hs@Wq, k/v = ehs@{Wk,Wv}, per-head
softmax((q k^T)/8 + col_bias) @ v, @Wo + bo + residual.  B=8 batches ->
data-parallel, one batch per NeuronCore (no collectives).

v2: fp8(e4m3, 240-max) DoubleRow matmuls for the two big GEMMs (q-projection
and the fused probs@(V*Wo) stack matmul) at 2x PE throughput; softmax
restructured so the Act engine writes exp() straight into the packed fp8
stack (no pack DMAs), denominators come from one batched 5-matmul fp8-DR
reduction, and the per-(head,q) normalization (with the "set-column"
suppression folded in exactly, via binary-mask algebra: exp(-20*(1-m)) ==
m up to 2e-9) is materialized by 10 selector matmuls + DVE muls.

Per-core dataflow:
  stage A (once): ehsT -> k,v [77,1024]; kT via PE transpose;
    M_h = v_h @ Wo_h packed (fp8) into m8 [128,10,1024].
  stage B (8 chunks of 512 q rows), software-pipelined:
    qT(ci+1) = Wq8^T @ hsT8 (fp8 DR, host-pretransposed hsT8) interleaved
    with scoresT(ci) = kT_h^T qT_h [77,512/head]; Act exp -> zstack pieces
    (fp8, packed layout [128,10,512]); D-batch: 5 DR matmuls with host
    selector blockonesX -> S0/SA/SB [48,512]; DVE: D = S0-(1-mA)SA-(1-mB)SB,
    reciprocal, dinvX [48,512] (rows 16-31 = dinv*mA, 32-47 = dinv*mB);
    db_kt = SelX_kt^T @ dinvX (10 matmuls) -> pstack = zstack*db (fp8);
    out = sum_ktpair pstack^T @ m8 (fp8 DR) + resid(bf16, bo folded in).

Inputs are the full unsharded arrays as produced by setup_inputs(); host side
shards/casts/transposes and builds the small selector/mask constants.
"""

import sys

for _p in ("/opt/trn_rl_repo",):
    if _p not in sys.path:
        sys.path.insert(0, _p)

import numpy as np
import ml_dtypes

import concourse.mybir as mybir
import concourse.tile as tile
from concourse import bacc
from concourse.bass import ds
from concourse.masks import make_identity

F32 = mybir.dt.float32
BF16 = mybir.dt.bfloat16
FP8 = mybir.dt.float8e4
AF = mybir.ActivationFunctionType
DR = mybir.MatmulPerfMode.DoubleRow

B, HW, C, CT, T, H, D = 8, 4096, 1024, 2048, 77, 16, 64
SUPPRESS = 20.0
SCL_Q = 32.0                  # fp8 range scaling for Wq, undone via Wk/SCL_Q
RT = H * T                    # 1232 stacked rows (16*77 head rows; no bo row)
NKT = (RT + 127) // 128       # 10 K-tiles for the AV matmul
NQ = 512                      # q rows per chunk
NCHUNK = HW // NQ             # 8


def _pack_pieces(h):
    """(tile_idx, part_base, src_start, nrows) pieces for head h's 77 rows at
    stacked row 77*h, split at 128-row tile boundaries."""
    g = T * h
    pieces = []
    pos = 0
    while pos < T:
        gg = g + pos
        ti, d = gg // 128, gg % 128
        n = min(T - pos, 128 - d)
        pieces.append((ti, d, pos, n))
        pos += n
    return pieces


def build_nc():
    nc = bacc.Bacc("TRN2", target_bir_lowering=False, debug=False)

    hsT8 = nc.dram_tensor("hsT8", [128, C // 128, HW], FP8, kind="ExternalInput")
    resid = nc.dram_tensor("resid", [128, NCHUNK, NQ // 128, C], BF16, kind="ExternalInput")
    ehsT = nc.dram_tensor("ehsT", [128, CT // 128, T], BF16, kind="ExternalInput")
    wq8 = nc.dram_tensor("wq8", [128, C // 128, C], FP8, kind="ExternalInput")
    wk = nc.dram_tensor("wk", [128, CT // 128, C], BF16, kind="ExternalInput")
    wv = nc.dram_tensor("wv", [128, CT // 128, C], BF16, kind="ExternalInput")
    wo = nc.dram_tensor("wo", [128, C // 128, C], BF16, kind="ExternalInput")
    bones = nc.dram_tensor("bones", [128, NKT, 96], FP8, kind="ExternalInput")
    selx = nc.dram_tensor("selx", [96, NKT, 128], BF16, kind="ExternalInput")
    masks = nc.dram_tensor("masks", [H, 4, HW], BF16, kind="ExternalInput")
    out = nc.dram_tensor("out", [128, NCHUNK, NQ // 128, C], BF16, kind="ExternalOutput")

    with tile.TileContext(nc) as tc:
        with (
            tc.tile_pool(name="const", bufs=1) as const,
            tc.tile_pool(name="persist", bufs=1) as persist,
        ):
            ident = const.tile([128, 128], BF16)
            make_identity(nc, ident)

            # resident constants (single-DMA loads)
            wq_sb = const.tile([128, C // 128, C], FP8)
            nc.sync.dma_start(wq_sb, wq8[:, :, :])
            bones_sb = const.tile([128, NKT, 96], FP8)
            nc.sync.dma_start(bones_sb, bones[:, :, :])
            selx_sb = const.tile([96, NKT, 128], BF16)
            nc.sync.dma_start(selx_sb, selx[:, :, :])
            masks_sb = const.tile([H, 4, HW], BF16)
            nc.sync.dma_start(masks_sb, masks[:, :, :])

            # persistent double-buffered stacks
            kT_sb = persist.tile([128, C // 128, T], BF16)
            m8 = persist.tile([128, NKT, C], FP8)
            nc.any.memset(m8, 0.0)
            zstacks = [persist.tile([128, NKT, NQ], FP8, name=f"zs{b}") for b in range(2)]
            pstacks = [persist.tile([128, NKT, NQ], FP8, name=f"ps{b}") for b in range(2)]
            dinvXs = [persist.tile([96, NQ], BF16, name=f"dx{b}") for b in range(2)]
            for dxb in dinvXs:
                nc.any.memset(dxb, 0.0)
            for zb, pb in zip(zstacks, pstacks):
                # rows past 1232 in tile 9 are never packed; zero so the
                # AV/D matmuls and muls see 0 (and never stale NaNs).
                nc.any.memset(zb[:, NKT - 1, :], 0.0)
                nc.any.memset(pb[:, NKT - 1, :], 0.0)

            # ---------------- stage A: k, v, kT, M ----------------
            with (
                tc.tile_pool(name="sa_sb", bufs=3) as sa_sb,
                tc.tile_pool(name="sa_ps", bufs=2, space="PSUM") as sa_ps,
            ):
                ehsT_sb = sa_sb.tile([128, CT // 128, T], BF16, bufs=1)
                nc.sync.dma_start(ehsT_sb, ehsT[:, :, :])
                wkv_sb = {}
                for name, wten in (("k", wk), ("v", wv)):
                    wt = sa_sb.tile([128, CT // 128, C], BF16, tag="wkv", bufs=2)
                    nc.sync.dma_start(wt, wten[:, :, :])
                    wkv_sb[name] = wt
                wo_sb = sa_sb.tile([128, C // 128, C], BF16, bufs=1)
                nc.sync.dma_start(wo_sb, wo[:, :, :])

                kv_sb = {}
                for name in ("k", "v"):
                    kv_ps = sa_ps.tile([T, C], F32, tag="kvps", bufs=1)
                    for j in range(CT // 128):
                        for nh in range(2):
                            nc.tensor.matmul(
                                kv_ps[:, ds(512 * nh, 512)],
                                ehsT_sb[:, j, :],
                                wkv_sb[name][:, j, ds(512 * nh, 512)],
                                start=(j == 0),
                                stop=(j == CT // 128 - 1),
                            )
                    kvs = sa_sb.tile([T, C], BF16, tag=f"{name}sb", bufs=1)
                    nc.any.tensor_copy(kvs, kv_ps)
                    kv_sb[name] = kvs

                # kT / vT via PE transpose of 128-column slices
                vT_sb = sa_sb.tile([128, C // 128, T], BF16, bufs=1)
                for src, dst in ((kv_sb["k"], kT_sb), (kv_sb["v"], vT_sb)):
                    for i in range(C // 128):
                        tp = sa_ps.tile([128, T], BF16, tag="tpa")
                        nc.tensor.transpose(tp, src[:, ds(128 * i, 128)], ident[:T, :T])
                        nc.any.tensor_copy(dst[:, i, :], tp)

                # M_h = v_h @ Wo_h, fp8, packed at stacked row 77h
                for h in range(H):
                    i, po = h // 2, (h % 2) * 64
                    m_ps = sa_ps.tile([T, C], F32, tag="mps")
                    for nh in range(2):
                        nc.tensor.matmul(
                            m_ps[:, ds(512 * nh, 512)],
                            vT_sb[ds(po, 64), i, :],
                            wo_sb[ds(po, 64), i, ds(512 * nh, 512)],
                            start=True,
                            stop=True,
                        )
                    m_stg = sa_sb.tile([T, C], FP8, tag="mstg")
                    nc.any.tensor_copy(m_stg, m_ps)
                    for (ti, pb, s0, nr) in _pack_pieces(h):
                        nc.gpsimd.dma_start(
                            m8[ds(pb, nr), ti, :], m_stg[ds(s0, nr), :]
                        )

            # ---------------- stage B: software-pipelined q chunks ----------------
            with (
                tc.tile_pool(name="io", bufs=2) as io,
                tc.tile_pool(name="work", bufs=2) as work,
                tc.tile_pool(name="ops", bufs=1, space="PSUM") as ops,
            ):
                st = {}

                def load(ci):
                    h8 = io.tile([128, C // 128, NQ], FP8, tag="h8")
                    nc.sync.dma_start(h8, hsT8[:, :, ds(NQ * ci, NQ)])
                    rs = io.tile([128, NQ // 128, C], BF16, tag="rs")
                    nc.sync.dma_start(rs, resid[:, ci, :, :])
                    qT = work.tile([128, C // 128, NQ], BF16, tag="qT")
                    z8 = work.tile([T, H, NQ], FP8, tag="z8")
                    st[ci] = dict(h8=h8, rs=rs, qT=qT, z8=z8, pk=0)

                def qt_group(ci, ij, use_act):
                    h8, qT = st[ci]["h8"], st[ci]["qT"]
                    q_ps = ops.tile([128, NQ], F32, tag="o128", bufs=3)
                    for jp in range(C // 256):
                        nc.tensor.matmul(
                            q_ps,
                            wq_sb[:, ds(2 * jp, 2), ds(128 * ij, 128)],
                            h8[:, ds(2 * jp, 2), :],
                            start=(jp == 0),
                            stop=(jp == C // 256 - 1),
                            perf_mode=DR,
                        )
                    if use_act:
                        nc.scalar.copy(qT[:, ij, :], q_ps)
                    else:
                        nc.vector.tensor_copy(qT[:, ij, :], q_ps)

                pack_q = [nc.gpsimd, nc.gpsimd, nc.gpsimd, nc.sync]

                def sm_head(ci, h):
                    qT = st[ci]["qT"]
                    zst = zstacks[ci % 2]
                    z8 = st[ci]["z8"]
                    i, po = h // 2, (h % 2) * 64
                    sT_ps = ops.tile([T, NQ], F32, tag="sT", bufs=2)
                    nc.tensor.matmul(
                        sT_ps,
                        kT_sb[ds(po, 64), i, :],
                        qT[ds(po, 64), i, :],
                        start=True,
                        stop=True,
                    )
                    nc.scalar.activation(z8[:, h, :], sT_ps, AF.Exp)
                    for (ti, pb, s0, nr) in _pack_pieces(h):
                        eng = pack_q[st[ci]["pk"] % len(pack_q)]
                        st[ci]["pk"] += 1
                        eng.dma_start(
                            zst[ds(pb, nr), ti, :], z8[ds(s0, nr), h, :]
                        )

                def d_chain(ci):
                    q0 = NQ * ci
                    zst = zstacks[ci % 2]
                    dX = dinvXs[ci % 2]
                    d_ps = ops.tile([96, NQ], F32, tag="dps", bufs=1)
                    for kp in range(NKT // 2):
                        nc.tensor.matmul(
                            d_ps,
                            bones_sb[:, ds(2 * kp, 2), :],
                            zst[:, ds(2 * kp, 2), :],
                            start=(kp == 0),
                            stop=(kp == NKT // 2 - 1),
                            perf_mode=DR,
                        )
                    # D = S0 - (1-mA)*SA - (1-mB)*SB
                    u = work.tile([H, NQ], F32, tag="u", bufs=2)
                    nc.vector.tensor_mul(u, d_ps[ds(32, H), :], masks_sb[:, 2, ds(q0, NQ)])
                    v = work.tile([H, NQ], F32, tag="v", bufs=2)
                    nc.vector.tensor_mul(v, d_ps[ds(64, H), :], masks_sb[:, 3, ds(q0, NQ)])
                    w = work.tile([H, NQ], F32, tag="w", bufs=2)
                    nc.vector.tensor_sub(w, d_ps[ds(0, H), :], u)
                    dd = work.tile([H, NQ], F32, tag="dd", bufs=2)
                    nc.vector.tensor_sub(dd, w, v)
                    dinv = work.tile([H, NQ], F32, tag="dinv", bufs=2)
                    nc.vector.reciprocal_approx_fast(dinv, dd)
                    nc.gpsimd.tensor_copy(dX[ds(0, H), :], dinv)
                    dA = work.tile([H, NQ], BF16, tag="dA", bufs=2)
                    nc.vector.tensor_mul(dA, dinv, masks_sb[:, 0, ds(q0, NQ)])
                    dB = work.tile([H, NQ], BF16, tag="dB", bufs=2)
                    nc.vector.tensor_mul(dB, dinv, masks_sb[:, 1, ds(q0, NQ)])
                    nc.gpsimd.dma_start(dX[ds(32, H), :], dA)
                    nc.gpsimd.dma_start(dX[ds(64, H), :], dB)

                def norm(ci):
                    zst, pst, dX = zstacks[ci % 2], pstacks[ci % 2], dinvXs[ci % 2]
                    for kt in range(NKT):
                        db_ps = ops.tile([128, NQ], F32, tag="db", bufs=2)
                        nc.tensor.matmul(
                            db_ps, selx_sb[:, kt, :], dX, start=True, stop=True
                        )
                        nc.vector.tensor_mul(pst[:, kt, :], zst[:, kt, :], db_ps)

                def av(ci):
                    pst, rs = pstacks[ci % 2], st[ci]["rs"]
                    for qj in range(NQ // 128):
                        o_sb = work.tile([128, 2, 512], BF16, tag="osb", bufs=3)
                        for nh in range(2):
                            o_ps = ops.tile([128, 512], F32, tag="o128", bufs=3)
                            for kp in range(NKT // 2):
                                nc.tensor.matmul(
                                    o_ps,
                                    pst[:, ds(2 * kp, 2), ds(128 * qj, 128)],
                                    m8[:, ds(2 * kp, 2), ds(512 * nh, 512)],
                                    start=(kp == 0),
                                    stop=False,
                                    perf_mode=DR,
                                )
                            # residual (+bo) accumulated via identity matmul
                            nc.tensor.matmul(
                                o_ps,
                                ident,
                                rs[:, qj, ds(512 * nh, 512)],
                                start=False,
                                stop=True,
                            )
                            if nh == 0:
                                nc.vector.tensor_copy(o_sb[:, nh, :], o_ps)
                            else:
                                nc.scalar.copy(o_sb[:, nh, :], o_ps)
                        nc.sync.dma_start(out[:, ci, qj, :], o_sb)

                load(0)
                for ij in range(C // 128):
                    qt_group(0, ij, ij % 2 == 0)
                for ci in range(NCHUNK):
                    if ci + 1 < NCHUNK:
                        load(ci + 1)
                    for h in range(H):
                        sm_head(ci, h)
                        if ci + 1 < NCHUNK and h % 2 == 1:
                            ij = h // 2
                            qt_group(ci + 1, ij, ij % 2 == 0)
                    d_chain(ci)
                    norm(ci)
                    av(ci)

    nc.compile()
    return nc


_NC_CACHE = {}


def get_nc():
    if "nc" not in _NC_CACHE:
        _NC_CACHE["nc"] = build_nc()
    return _NC_CACHE["nc"]


def _bf(x):
    return np.asarray(x, dtype=ml_dtypes.bfloat16)


def _f8(x):
    return np.asarray(x, dtype=ml_dtypes.float8_e4m3)


def make_in_maps(inputs):
    hs = np.asarray(inputs["hidden_states"], dtype=np.float32)
    ehs = np.asarray(inputs["encoder_hidden_states"], dtype=np.float32)
    mask_A = np.asarray(inputs["mask_A"], dtype=np.float32)
    mask_B = np.asarray(inputs["mask_B"], dtype=np.float32)
    Wq = np.asarray(inputs["Wq"], dtype=np.float32)
    Wk = np.asarray(inputs["Wk"], dtype=np.float32)
    Wv = np.asarray(inputs["Wv"], dtype=np.float32)
    Wo = np.asarray(inputs["Wo"], dtype=np.float32)
    bo = np.asarray(inputs["bo"], dtype=np.float32)
    idxA = list(np.asarray(inputs["token_indices_A"]).astype(np.int64) % T)
    idxB = list(np.asarray(inputs["token_indices_B"]).astype(np.int64) % T)
    setA = [t for t in idxA if t not in idxB]
    setB = list(dict.fromkeys(idxB))

    scale = 1.0 / np.sqrt(D)
    wq8 = _f8(Wq * (scale * SCL_Q))            # [C, C]
    wq8 = np.ascontiguousarray(
        wq8.reshape(C // 128, 128, C).transpose(1, 0, 2)
    )                                          # [128, 8, C]
    wk_bf = _bf(Wk * (1.0 / SCL_Q)).reshape(CT // 128, 128, C).transpose(1, 0, 2)
    wv_bf = _bf(Wv).reshape(CT // 128, 128, C).transpose(1, 0, 2)
    wo_bf = _bf(Wo).reshape(C // 128, 128, C).transpose(1, 0, 2)
    wk_bf = np.ascontiguousarray(wk_bf)
    wv_bf = np.ascontiguousarray(wv_bf)
    wo_bf = np.ascontiguousarray(wo_bf)

    # blockonesX [128, NKT, 48] fp8 and SelX [48, NKT, 128] bf16
    bones = np.zeros((128, NKT, 96), np.float32)
    sel = np.zeros((96, NKT, 128), np.float32)
    for r in range(RT):
        kt, p = r // 128, r % 128
        h, t = r // T, r % T
        bones[p, kt, h] = 1.0                   # S0: all t
        if t in setA:
            bones[p, kt, 32 + h] = 1.0          # SA
            sel[32 + h, kt, p] = 1.0            # dinv*mA
        elif t in setB:
            bones[p, kt, 64 + h] = 1.0          # SB
            sel[64 + h, kt, p] = 1.0            # dinv*mB
        else:
            sel[h, kt, p] = 1.0                 # dinv
    bones8 = _f8(bones)
    sel_bf = _bf(sel)

    masks = np.zeros((H, 4, HW), np.float32)
    masks[:, 0] = mask_A[None, :]
    masks[:, 1] = mask_B[None, :]
    masks[:, 2] = (1.0 - mask_A)[None, :]
    masks[:, 3] = (1.0 - mask_B)[None, :]
    masks_bf = _bf(masks)

    ehsT_l = None
    in_maps = []
    for b in range(B):
        hsb = hs[b]
        hs_bf = _bf(hsb)
        hsT8 = _f8(hs_bf.astype(np.float32).T)             # [C, HW]
        hsT8 = np.ascontiguousarray(
            hsT8.reshape(C // 128, 128, HW).transpose(1, 0, 2)
        )                                                   # [128, 8, HW]
        residb = _bf(hsb + bo[None, :])                     # [HW, C]
        residb = np.ascontiguousarray(
            residb.reshape(NCHUNK, NQ // 128, 128, C).transpose(2, 0, 1, 3)
        )                                                   # [128, 8, 4, C]
        ehsT_l = _bf(ehs[b].T.reshape(CT // 128, 128, T).transpose(1, 0, 2))
        in_maps.append(
            {
                "hsT8": hsT8,
                "resid": residb,
                "ehsT": np.ascontiguousarray(ehsT_l),
                "wq8": wq8,
                "wk": wk_bf,
                "wv": wv_bf,
                "wo": wo_bf,
                "bones": bones8,
                "selx": sel_bf,
                "masks": masks_bf,
            }
        )
    return in_maps


def unpack_out(arr):
    # arr [128, NCHUNK, NQ//128, C] -> [HW, C]
    return np.ascontiguousarray(
        np.asarray(arr).transpose(1, 2, 0, 3).reshape(HW, C)
    )


def kernel(**inputs) -> np.ndarray:
    from concourse.bass_utils import run_bass_kernel_spmd

    nc = get_nc()
    in_maps = make_in_maps(inputs)
    res = run_bass_kernel_spmd(nc, in_maps, core_ids=list(range(B)))
    return np.stack([unpack_out(res.results[b]["out"]) for b in range(B)]).astype(
        np.float32
    )


# revision 15
# speedup vs baseline: 1.0287x; 1.0287x over previous
"""Trainium2 Bass kernel for nn_DenseAttnProcessor (sparse_attention).

Cross-attention block: q = hs@Wq, k/v = ehs@{Wk,Wv}, per-head
softmax((q k^T)/8 + col_bias) @ v, @Wo + bo + residual.  B=8 batches ->
data-parallel, one batch per NeuronCore (no collectives).

v2: fp8(e4m3, 240-max) DoubleRow matmuls for the two big GEMMs (q-projection
and the fused probs@(V*Wo) stack matmul) at 2x PE throughput; softmax
restructured so the Act engine writes exp() straight into the packed fp8
stack (no pack DMAs), denominators come from one batched 5-matmul fp8-DR
reduction, and the per-(head,q) normalization (with the "set-column"
suppression folded in exactly, via binary-mask algebra: exp(-20*(1-m)) ==
m up to 2e-9) is materialized by 10 selector matmuls + DVE muls.

Per-core dataflow:
  stage A (once): ehsT -> k,v [77,1024]; kT via PE transpose;
    M_h = v_h @ Wo_h packed (fp8) into m8 [128,10,1024].
  stage B (8 chunks of 512 q rows), software-pipelined:
    qT(ci+1) = Wq8^T @ hsT8 (fp8 DR, host-pretransposed hsT8) interleaved
    with scoresT(ci) = kT_h^T qT_h [77,512/head]; Act exp -> zstack pieces
    (fp8, packed layout [128,10,512]); D-batch: 5 DR matmuls with host
    selector blockonesX -> S0/SA/SB [48,512]; DVE: D = S0-(1-mA)SA-(1-mB)SB,
    reciprocal, dinvX [48,512] (rows 16-31 = dinv*mA, 32-47 = dinv*mB);
    db_kt = SelX_kt^T @ dinvX (10 matmuls) -> pstack = zstack*db (fp8);
    out = sum_ktpair pstack^T @ m8 (fp8 DR) + resid(bf16, bo folded in).

Inputs are the full unsharded arrays as produced by setup_inputs(); host side
shards/casts/transposes and builds the small selector/mask constants.
"""

import sys

for _p in ("/opt/trn_rl_repo",):
    if _p not in sys.path:
        sys.path.insert(0, _p)

import numpy as np
import ml_dtypes

import concourse.mybir as mybir
import concourse.tile as tile
from concourse import bacc
from concourse.bass import ds
from concourse.masks import make_identity

F32 = mybir.dt.float32
BF16 = mybir.dt.bfloat16
FP8 = mybir.dt.float8e4
AF = mybir.ActivationFunctionType
DR = mybir.MatmulPerfMode.DoubleRow

B, HW, C, CT, T, H, D = 8, 4096, 1024, 2048, 77, 16, 64
SUPPRESS = 20.0
SCL_Q = 32.0                  # fp8 range scaling for Wq, undone via Wk/SCL_Q
RT = H * T                    # 1232 stacked rows (16*77 head rows; no bo row)
NKT = (RT + 127) // 128       # 10 K-tiles for the AV matmul
NQ = 512                      # q rows per chunk
NCHUNK = HW // NQ             # 8


def _pack_pieces(h):
    """(tile_idx, part_base, src_start, nrows) pieces for head h's 77 rows at
    stacked row 77*h, split at 128-row tile boundaries."""
    g = T * h
    pieces = []
    pos = 0
    while pos < T:
        gg = g + pos
        ti, d = gg // 128, gg % 128
        n = min(T - pos, 128 - d)
        pieces.append((ti, d, pos, n))
        pos += n
    return pieces


def build_nc():
    nc = bacc.Bacc("TRN2", target_bir_lowering=False, debug=False)

    hsT8 = nc.dram_tensor("hsT8", [128, C // 128, HW], FP8, kind="ExternalInput")
    resid = nc.dram_tensor("resid", [128, NCHUNK, NQ // 128, C], BF16, kind="ExternalInput")
    ehsT = nc.dram_tensor("ehsT", [128, CT // 128, T], BF16, kind="ExternalInput")
    wq8 = nc.dram_tensor("wq8", [128, C // 128, C], FP8, kind="ExternalInput")
    wk = nc.dram_tensor("wk", [128, CT // 128, C], BF16, kind="ExternalInput")
    wv = nc.dram_tensor("wv", [128, CT // 128, C], BF16, kind="ExternalInput")
    wo = nc.dram_tensor("wo", [128, C // 128, C], BF16, kind="ExternalInput")
    bones = nc.dram_tensor("bones", [128, NKT, 96], FP8, kind="ExternalInput")
    selx = nc.dram_tensor("selx", [96, NKT, 128], BF16, kind="ExternalInput")
    masks = nc.dram_tensor("masks", [H, 4, HW], BF16, kind="ExternalInput")
    out = nc.dram_tensor("out", [128, NCHUNK, NQ // 128, C], BF16, kind="ExternalOutput")

    with tile.TileContext(nc) as tc:
        with (
            tc.tile_pool(name="const", bufs=1) as const,
            tc.tile_pool(name="persist", bufs=1) as persist,
        ):
            ident = const.tile([128, 128], BF16)
            make_identity(nc, ident)

            # resident constants (single-DMA loads)
            wq_sb = const.tile([128, C // 128, C], FP8)
            nc.gpsimd.dma_start(wq_sb, wq8[:, :, :])
            bones_sb = const.tile([128, NKT, 96], FP8)
            nc.gpsimd.dma_start(bones_sb, bones[:, :, :])
            selx_sb = const.tile([96, NKT, 128], BF16)
            nc.gpsimd.dma_start(selx_sb, selx[:, :, :])
            masks_sb = const.tile([H, 4, HW], BF16)
            for mj in range(4):
                nc.scalar.dma_start(masks_sb[:, mj, :], masks[:, mj, :])

            # persistent double-buffered stacks
            kT_sb = persist.tile([128, C // 128, T], BF16)
            m8 = persist.tile([128, NKT, C], FP8)
            nc.any.memset(m8, 0.0)
            zstacks = [persist.tile([128, NKT, NQ], FP8, name=f"zs{b}") for b in range(2)]
            pstacks = [persist.tile([128, NKT, NQ], FP8, name=f"ps{b}") for b in range(2)]
            dinvXs = [persist.tile([96, NQ], BF16, name=f"dx{b}") for b in range(2)]
            for dxb in dinvXs:
                nc.any.memset(dxb, 0.0)
            for zb, pb in zip(zstacks, pstacks):
                # rows past 1232 in tile 9 are never packed; zero so the
                # AV/D matmuls and muls see 0 (and never stale NaNs).
                nc.any.memset(zb[:, NKT - 1, :], 0.0)
                nc.any.memset(pb[:, NKT - 1, :], 0.0)

            # ---------------- stage A: k, v, kT, M ----------------
            with (
                tc.tile_pool(name="sa_sb", bufs=3) as sa_sb,
                tc.tile_pool(name="sa_ps", bufs=2, space="PSUM") as sa_ps,
            ):
                ehsT_sb = sa_sb.tile([128, CT // 128, T], BF16, bufs=1)
                nc.sync.dma_start(ehsT_sb, ehsT[:, :, :])
                wkv_sb = {}
                for name, wten, q0, q1 in (
                    ("k", wk, nc.sync, nc.scalar),
                    ("v", wv, nc.scalar, nc.gpsimd),
                ):
                    wt = sa_sb.tile([128, CT // 128, C], BF16, tag="wkv", bufs=2)
                    nj = CT // 256
                    q0.dma_start(wt[:, ds(0, nj), :], wten[:, ds(0, nj), :])
                    q1.dma_start(wt[:, ds(nj, nj), :], wten[:, ds(nj, nj), :])
                    wkv_sb[name] = wt
                wo_sb = sa_sb.tile([128, C // 128, C], BF16, bufs=1)
                nc.gpsimd.dma_start(wo_sb[:, ds(0, 4), :], wo[:, ds(0, 4), :])
                nc.sync.dma_start(wo_sb[:, ds(4, 4), :], wo[:, ds(4, 4), :])

                kv_sb = {}
                for name in ("k", "v"):
                    kv_ps = sa_ps.tile([T, C], F32, tag="kvps", bufs=1)
                    for j in range(CT // 128):
                        for nh in range(2):
                            nc.tensor.matmul(
                                kv_ps[:, ds(512 * nh, 512)],
                                ehsT_sb[:, j, :],
                                wkv_sb[name][:, j, ds(512 * nh, 512)],
                                start=(j == 0),
                                stop=(j == CT // 128 - 1),
                            )
                    kvs = sa_sb.tile([T, C], BF16, tag=f"{name}sb", bufs=1)
                    nc.any.tensor_copy(kvs, kv_ps)
                    kv_sb[name] = kvs

                # kT / vT via PE transpose of 128-column slices
                vT_sb = sa_sb.tile([128, C // 128, T], BF16, bufs=1)
                for src, dst in ((kv_sb["k"], kT_sb), (kv_sb["v"], vT_sb)):
                    for i in range(C // 128):
                        tp = sa_ps.tile([128, T], BF16, tag="tpa")
                        nc.tensor.transpose(tp, src[:, ds(128 * i, 128)], ident[:T, :T])
                        nc.any.tensor_copy(dst[:, i, :], tp)

                # M_h = v_h @ Wo_h, fp8, packed at stacked row 77h
                for h in range(H):
                    i, po = h // 2, (h % 2) * 64
                    m_ps = sa_ps.tile([T, C], F32, tag="mps")
                    for nh in range(2):
                        nc.tensor.matmul(
                            m_ps[:, ds(512 * nh, 512)],
                            vT_sb[ds(po, 64), i, :],
                            wo_sb[ds(po, 64), i, ds(512 * nh, 512)],
                            start=True,
                            stop=True,
                        )
                    m_stg = sa_sb.tile([T, C], FP8, tag="mstg")
                    nc.any.tensor_copy(m_stg, m_ps)
                    for (ti, pb, s0, nr) in _pack_pieces(h):
                        nc.gpsimd.dma_start(
                            m8[ds(pb, nr), ti, :], m_stg[ds(s0, nr), :]
                        )

            # ---------------- stage B: software-pipelined q chunks ----------------
            with (
                tc.tile_pool(name="io", bufs=2) as io,
                tc.tile_pool(name="work", bufs=2) as work,
                tc.tile_pool(name="ops", bufs=1, space="PSUM") as ops,
            ):
                st = {}

                def load(ci):
                    h8 = io.tile([128, C // 128, NQ], FP8, tag="h8")
                    nc.sync.dma_start(h8, hsT8[:, :, ds(NQ * ci, NQ)])
                    rs = io.tile([128, NQ // 128, C], BF16, tag="rs", bufs=3)
                    nc.sync.dma_start(rs, resid[:, ci, :, :])
                    qT = work.tile([128, C // 128, NQ], BF16, tag="qT")
                    z8 = work.tile([T, H, NQ], FP8, tag="z8")
                    st[ci] = dict(h8=h8, rs=rs, qT=qT, z8=z8, pk=0)

                def qt_group(ci, ij, use_act):
                    h8, qT = st[ci]["h8"], st[ci]["qT"]
                    q_ps = ops.tile([128, NQ], F32, tag="o128", bufs=3)
                    for jp in range(C // 256):
                        nc.tensor.matmul(
                            q_ps,
                            wq_sb[:, ds(2 * jp, 2), ds(128 * ij, 128)],
                            h8[:, ds(2 * jp, 2), :],
                            start=(jp == 0),
                            stop=(jp == C // 256 - 1),
                            perf_mode=DR,
                        )
                    if use_act:
                        nc.scalar.copy(qT[:, ij, :], q_ps)
                    else:
                        nc.vector.tensor_copy(qT[:, ij, :], q_ps)

                pack_q = [nc.gpsimd, nc.gpsimd, nc.gpsimd, nc.sync]

                def sm_head(ci, h):
                    qT = st[ci]["qT"]
                    zst = zstacks[ci % 2]
                    z8 = st[ci]["z8"]
                    i, po = h // 2, (h % 2) * 64
                    sT_ps = ops.tile([T, NQ], F32, tag="sT", bufs=2)
                    nc.tensor.matmul(
                        sT_ps,
                        kT_sb[ds(po, 64), i, :],
                        qT[ds(po, 64), i, :],
                        start=True,
                        stop=True,
                    )
                    nc.scalar.activation(z8[:, h, :], sT_ps, AF.Exp)
                    for (ti, pb, s0, nr) in _pack_pieces(h):
                        eng = pack_q[st[ci]["pk"] % len(pack_q)]
                        st[ci]["pk"] += 1
                        eng.dma_start(
                            zst[ds(pb, nr), ti, :], z8[ds(s0, nr), h, :]
                        )

                def d_chain(ci):
                    q0 = NQ * ci
                    zst = zstacks[ci % 2]
                    dX = dinvXs[ci % 2]
                    d_ps = ops.tile([96, NQ], F32, tag="dps", bufs=1)
                    for kp in range(NKT // 2):
                        nc.tensor.matmul(
                            d_ps,
                            bones_sb[:, ds(2 * kp, 2), :],
                            zst[:, ds(2 * kp, 2), :],
                            start=(kp == 0),
                            stop=(kp == NKT // 2 - 1),
                            perf_mode=DR,
                        )
                    # D = S0 - (1-mA)*SA - (1-mB)*SB
                    u = work.tile([H, NQ], F32, tag="u", bufs=2)
                    nc.vector.tensor_mul(u, d_ps[ds(32, H), :], masks_sb[:, 2, ds(q0, NQ)])
                    v = work.tile([H, NQ], F32, tag="v", bufs=2)
                    nc.vector.tensor_mul(v, d_ps[ds(64, H), :], masks_sb[:, 3, ds(q0, NQ)])
                    w = work.tile([H, NQ], F32, tag="w", bufs=2)
                    nc.vector.tensor_sub(w, d_ps[ds(0, H), :], u)
                    dd = work.tile([H, NQ], F32, tag="dd", bufs=2)
                    nc.vector.tensor_sub(dd, w, v)
                    dinv = work.tile([H, NQ], F32, tag="dinv", bufs=2)
                    nc.vector.reciprocal_approx_fast(dinv, dd)
                    nc.gpsimd.tensor_copy(dX[ds(0, H), :], dinv)
                    dA = work.tile([H, NQ], BF16, tag="dA", bufs=2)
                    nc.vector.tensor_mul(dA, dinv, masks_sb[:, 0, ds(q0, NQ)])
                    dB = work.tile([H, NQ], BF16, tag="dB", bufs=2)
                    nc.vector.tensor_mul(dB, dinv, masks_sb[:, 1, ds(q0, NQ)])
                    nc.gpsimd.dma_start(dX[ds(32, H), :], dA)
                    nc.gpsimd.dma_start(dX[ds(64, H), :], dB)

                def norm(ci):
                    zst, pst, dX = zstacks[ci % 2], pstacks[ci % 2], dinvXs[ci % 2]
                    for kt in range(NKT):
                        db_ps = ops.tile([128, NQ], F32, tag="db", bufs=2)
                        nc.tensor.matmul(
                            db_ps, selx_sb[:, kt, :], dX, start=True, stop=True
                        )
                        nc.vector.tensor_mul(pst[:, kt, :], zst[:, kt, :], db_ps)

                def av(ci):
                    pst, rs = pstacks[ci % 2], st[ci]["rs"]
                    for qj in range(NQ // 128):
                        o_sb = work.tile([128, 2, 512], BF16, tag="osb", bufs=3)
                        for nh in range(2):
                            o_ps = ops.tile([128, 512], F32, tag="o128", bufs=3)
                            for kp in range(NKT // 2):
                                nc.tensor.matmul(
                                    o_ps,
                                    pst[:, ds(2 * kp, 2), ds(128 * qj, 128)],
                                    m8[:, ds(2 * kp, 2), ds(512 * nh, 512)],
                                    start=(kp == 0),
                                    stop=False,
                                    perf_mode=DR,
                                )
                            # residual (+bo) accumulated via identity matmul
                            nc.tensor.matmul(
                                o_ps,
                                ident,
                                rs[:, qj, ds(512 * nh, 512)],
                                start=False,
                                stop=True,
                            )
                            if nh == 0:
                                nc.vector.tensor_copy(o_sb[:, nh, :], o_ps)
                            else:
                                nc.scalar.copy(o_sb[:, nh, :], o_ps)
                        nc.sync.dma_start(out[:, ci, qj, :], o_sb)

                load(0)
                for ij in range(C // 128):
                    qt_group(0, ij, ij % 2 == 0)
                for ci in range(NCHUNK):
                    if ci + 1 < NCHUNK:
                        load(ci + 1)
                    for h in range(H):
                        sm_head(ci, h)
                        if ci + 1 < NCHUNK and h % 2 == 1:
                            ij = h // 2
                            qt_group(ci + 1, ij, ij % 2 == 0)
                    d_chain(ci)
                    if ci > 0:
                        av(ci - 1)   # hides the DVE d-chain latency
                    norm(ci)
                av(NCHUNK - 1)

    nc.compile()
    return nc


_NC_CACHE = {}


def get_nc():
    if "nc" not in _NC_CACHE:
        _NC_CACHE["nc"] = build_nc()
    return _NC_CACHE["nc"]


def _bf(x):
    return np.asarray(x, dtype=ml_dtypes.bfloat16)


def _f8(x):
    return np.asarray(x, dtype=ml_dtypes.float8_e4m3)


def make_in_maps(inputs):
    hs = np.asarray(inputs["hidden_states"], dtype=np.float32)
    ehs = np.asarray(inputs["encoder_hidden_states"], dtype=np.float32)
    mask_A = np.asarray(inputs["mask_A"], dtype=np.float32)
    mask_B = np.asarray(inputs["mask_B"], dtype=np.float32)
    Wq = np.asarray(inputs["Wq"], dtype=np.float32)
    Wk = np.asarray(inputs["Wk"], dtype=np.float32)
    Wv = np.asarray(inputs["Wv"], dtype=np.float32)
    Wo = np.asarray(inputs["Wo"], dtype=np.float32)
    bo = np.asarray(inputs["bo"], dtype=np.float32)
    idxA = list(np.asarray(inputs["token_indices_A"]).astype(np.int64) % T)
    idxB = list(np.asarray(inputs["token_indices_B"]).astype(np.int64) % T)
    setA = [t for t in idxA if t not in idxB]
    setB = list(dict.fromkeys(idxB))

    scale = 1.0 / np.sqrt(D)
    wq8 = _f8(Wq * (scale * SCL_Q))            # [C, C]
    wq8 = np.ascontiguousarray(
        wq8.reshape(C // 128, 128, C).transpose(1, 0, 2)
    )                                          # [128, 8, C]
    wk_bf = _bf(Wk * (1.0 / SCL_Q)).reshape(CT // 128, 128, C).transpose(1, 0, 2)
    wv_bf = _bf(Wv).reshape(CT // 128, 128, C).transpose(1, 0, 2)
    wo_bf = _bf(Wo).reshape(C // 128, 128, C).transpose(1, 0, 2)
    wk_bf = np.ascontiguousarray(wk_bf)
    wv_bf = np.ascontiguousarray(wv_bf)
    wo_bf = np.ascontiguousarray(wo_bf)

    # blockonesX [128, NKT, 48] fp8 and SelX [48, NKT, 128] bf16
    bones = np.zeros((128, NKT, 96), np.float32)
    sel = np.zeros((96, NKT, 128), np.float32)
    for r in range(RT):
        kt, p = r // 128, r % 128
        h, t = r // T, r % T
        bones[p, kt, h] = 1.0                   # S0: all t
        if t in setA:
            bones[p, kt, 32 + h] = 1.0          # SA
            sel[32 + h, kt, p] = 1.0            # dinv*mA
        elif t in setB:
            bones[p, kt, 64 + h] = 1.0          # SB
            sel[64 + h, kt, p] = 1.0            # dinv*mB
        else:
            sel[h, kt, p] = 1.0                 # dinv
    bones8 = _f8(bones)
    sel_bf = _bf(sel)

    masks = np.zeros((H, 4, HW), np.float32)
    masks[:, 0] = mask_A[None, :]
    masks[:, 1] = mask_B[None, :]
    masks[:, 2] = (1.0 - mask_A)[None, :]
    masks[:, 3] = (1.0 - mask_B)[None, :]
    masks_bf = _bf(masks)

    ehsT_l = None
    in_maps = []
    for b in range(B):
        hsb = hs[b]
        hs_bf = _bf(hsb)
        hsT8 = _f8(hs_bf.astype(np.float32).T)             # [C, HW]
        hsT8 = np.ascontiguousarray(
            hsT8.reshape(C // 128, 128, HW).transpose(1, 0, 2)
        )                                                   # [128, 8, HW]
        residb = _bf(hsb + bo[None, :])                     # [HW, C]
        residb = np.ascontiguousarray(
            residb.reshape(NCHUNK, NQ // 128, 128, C).transpose(2, 0, 1, 3)
        )                                                   # [128, 8, 4, C]
        ehsT_l = _bf(ehs[b].T.reshape(CT // 128, 128, T).transpose(1, 0, 2))
        in_maps.append(
            {
                "hsT8": hsT8,
                "resid": residb,
                "ehsT": np.ascontiguousarray(ehsT_l),
                "wq8": wq8,
                "wk": wk_bf,
                "wv": wv_bf,
                "wo": wo_bf,
                "bones": bones8,
                "selx": sel_bf,
                "masks": masks_bf,
            }
        )
    return in_maps


def unpack_out(arr):
    # arr [128, NCHUNK, NQ//128, C] -> [HW, C]
    return np.ascontiguousarray(
        np.asarray(arr).transpose(1, 2, 0, 3).reshape(HW, C)
    )


def kernel(**inputs) -> np.ndarray:
    from concourse.bass_utils import run_bass_kernel_spmd

    nc = get_nc()
    in_maps = make_in_maps(inputs)
    res = run_bass_kernel_spmd(nc, in_maps, core_ids=list(range(B)))
    return np.stack([unpack_out(res.results[b]["out"]) for b in range(B)]).astype(
        np.float32
    )


# revision 17
# speedup vs baseline: 1.0511x; 1.0217x over previous
"""Trainium2 Bass kernel for nn_DenseAttnProcessor (sparse_attention).

Cross-attention block: q = hs@Wq, k/v = ehs@{Wk,Wv}, per-head
softmax((q k^T)/8 + col_bias) @ v, @Wo + bo + residual.  B=8 batches ->
data-parallel, one batch per NeuronCore (no collectives).

v2: fp8(e4m3, 240-max) DoubleRow matmuls for the two big GEMMs (q-projection
and the fused probs@(V*Wo) stack matmul) at 2x PE throughput; softmax
restructured so the Act engine writes exp() straight into the packed fp8
stack (no pack DMAs), denominators come from one batched 5-matmul fp8-DR
reduction, and the per-(head,q) normalization (with the "set-column"
suppression folded in exactly, via binary-mask algebra: exp(-20*(1-m)) ==
m up to 2e-9) is materialized by 10 selector matmuls + DVE muls.

Per-core dataflow:
  stage A (once): ehsT -> k,v [77,1024]; kT via PE transpose;
    M_h = v_h @ Wo_h packed (fp8) into m8 [128,10,1024].
  stage B (8 chunks of 512 q rows), software-pipelined:
    qT(ci+1) = Wq8^T @ hsT8 (fp8 DR, host-pretransposed hsT8) interleaved
    with scoresT(ci) = kT_h^T qT_h [77,512/head]; Act exp -> zstack pieces
    (fp8, packed layout [128,10,512]); D-batch: 5 DR matmuls with host
    selector blockonesX -> S0/SA/SB [48,512]; DVE: D = S0-(1-mA)SA-(1-mB)SB,
    reciprocal, dinvX [48,512] (rows 16-31 = dinv*mA, 32-47 = dinv*mB);
    db_kt = SelX_kt^T @ dinvX (10 matmuls) -> pstack = zstack*db (fp8);
    out = sum_ktpair pstack^T @ m8 (fp8 DR) + resid(bf16, bo folded in).

Inputs are the full unsharded arrays as produced by setup_inputs(); host side
shards/casts/transposes and builds the small selector/mask constants.
"""

import sys

for _p in ("/opt/trn_rl_repo",):
    if _p not in sys.path:
        sys.path.insert(0, _p)

import numpy as np
import ml_dtypes

import concourse.mybir as mybir
import concourse.tile as tile
from concourse import bacc
from concourse.bass import ds
from concourse.masks import make_identity

F32 = mybir.dt.float32
BF16 = mybir.dt.bfloat16
FP8 = mybir.dt.float8e4
AF = mybir.ActivationFunctionType
DR = mybir.MatmulPerfMode.DoubleRow

B, HW, C, CT, T, H, D = 8, 4096, 1024, 2048, 77, 16, 64
SUPPRESS = 20.0
SCL_Q = 32.0                  # fp8 range scaling for Wq, undone via Wk/SCL_Q
RT = H * T                    # 1232 stacked rows (16*77 head rows; no bo row)
NKT = (RT + 127) // 128       # 10 K-tiles for the AV matmul
NQ = 512                      # q rows per chunk
NCHUNK = HW // NQ             # 8


def _pack_pieces(h):
    """(tile_idx, part_base, src_start, nrows) pieces for head h's 77 rows at
    stacked row 77*h, split at 128-row tile boundaries."""
    g = T * h
    pieces = []
    pos = 0
    while pos < T:
        gg = g + pos
        ti, d = gg // 128, gg % 128
        n = min(T - pos, 128 - d)
        pieces.append((ti, d, pos, n))
        pos += n
    return pieces


def build_nc():
    nc = bacc.Bacc("TRN2", target_bir_lowering=False, debug=False)

    hsT8 = nc.dram_tensor("hsT8", [128, C // 128, HW], FP8, kind="ExternalInput")
    resid = nc.dram_tensor("resid", [128, NCHUNK, NQ // 128, C], BF16, kind="ExternalInput")
    ehsT = nc.dram_tensor("ehsT", [128, CT // 128, T], BF16, kind="ExternalInput")
    wq8 = nc.dram_tensor("wq8", [128, C // 128, C], FP8, kind="ExternalInput")
    wk = nc.dram_tensor("wk", [128, CT // 128, C], BF16, kind="ExternalInput")
    wv = nc.dram_tensor("wv", [128, CT // 128, C], BF16, kind="ExternalInput")
    wo = nc.dram_tensor("wo", [128, C // 128, C], BF16, kind="ExternalInput")
    bones = nc.dram_tensor("bones", [128, NKT, 96], FP8, kind="ExternalInput")
    selx = nc.dram_tensor("selx", [96, NKT, 128], BF16, kind="ExternalInput")
    masks = nc.dram_tensor("masks", [H, 4, HW], BF16, kind="ExternalInput")
    out = nc.dram_tensor("out", [128, NCHUNK, NQ // 128, C], BF16, kind="ExternalOutput")

    with tile.TileContext(nc) as tc:
        with (
            tc.tile_pool(name="const", bufs=1) as const,
            tc.tile_pool(name="persist", bufs=1) as persist,
            tc.tile_pool(name="sa", bufs=1) as sa,
            tc.tile_pool(name="io", bufs=2) as io,
            tc.tile_pool(name="work", bufs=2) as work,
            tc.tile_pool(name="ops", bufs=1, space="PSUM") as ops,
        ):
            st = {}

            def load(ci):
                h8 = io.tile([128, C // 128, NQ], FP8, tag="h8")
                nc.sync.dma_start(h8, hsT8[:, :, ds(NQ * ci, NQ)])
                rs = io.tile([128, NQ // 128, C], BF16, tag="rs", bufs=3)
                nc.sync.dma_start(rs, resid[:, ci, :, :])
                qT = work.tile([128, C // 128, NQ], BF16, tag="qT")
                z8 = work.tile([T, H, NQ], FP8, tag="z8")
                mk = io.tile([H, 4, NQ], BF16, tag="mk")
                nc.scalar.dma_start(mk, masks[:, :, ds(NQ * ci, NQ)])
                st[ci] = dict(h8=h8, rs=rs, qT=qT, z8=z8, mk=mk, pk=0)

            # ---- prefetches: chunk 0 + all weights, spread over queues ----
            wq_sb = const.tile([128, C // 128, C], FP8)
            nc.gpsimd.dma_start(wq_sb, wq8[:, :, :])
            load(0)
            ehsT_sb = sa.tile([128, CT // 128, T], BF16)
            nc.sync.dma_start(ehsT_sb, ehsT[:, :, :])

            wo_sb = sa.tile([128, C // 128, C], BF16)
            nc.gpsimd.dma_start(wo_sb[:, ds(0, 4), :], wo[:, ds(0, 4), :])
            nc.sync.dma_start(wo_sb[:, ds(4, 4), :], wo[:, ds(4, 4), :])
            bones_sb = const.tile([128, NKT, 96], FP8)
            nc.gpsimd.dma_start(bones_sb, bones[:, :, :])
            selx_sb = const.tile([96, NKT, 128], BF16)
            nc.gpsimd.dma_start(selx_sb, selx[:, :, :])


            ident = const.tile([128, 128], BF16)
            make_identity(nc, ident)
            identf = const.tile([128, 128], F32)
            make_identity(nc, identf)

            # persistent stacks
            kT_sb = persist.tile([128, C // 128, T], BF16)
            vT_sb = persist.tile([128, C // 128, T], BF16)
            m8 = persist.tile([128, NKT, C], FP8)
            nc.any.memset(m8[:, NKT - 1, :], 0.0)
            zstacks = [persist.tile([128, NKT, NQ], FP8, name=f"zs{b}") for b in range(2)]
            pstacks = [persist.tile([128, NKT, NQ], FP8, name=f"ps{b}") for b in range(2)]
            dinvXs = [persist.tile([96, NQ], BF16, name=f"dx{b}") for b in range(2)]
            for dxb in dinvXs:
                nc.any.memset(dxb, 0.0)
            for zb, pb in zip(zstacks, pstacks):
                nc.any.memset(zb[:, NKT - 1, :], 0.0)
                nc.any.memset(pb[:, NKT - 1, :], 0.0)

            def qt_group(ci, ij, use_act):
                h8, qT = st[ci]["h8"], st[ci]["qT"]
                q_ps = ops.tile([128, NQ], F32, tag="o128", bufs=3)
                for jp in range(C // 256):
                    nc.tensor.matmul(
                        q_ps,
                        wq_sb[:, ds(2 * jp, 2), ds(128 * ij, 128)],
                        h8[:, ds(2 * jp, 2), :],
                        start=(jp == 0),
                        stop=(jp == C // 256 - 1),
                        perf_mode=DR,
                    )
                if use_act:
                    nc.scalar.copy(qT[:, ij, :], q_ps)
                else:
                    nc.vector.tensor_copy(qT[:, ij, :], q_ps)

            # ---- stage A (overlaps chunk-0 qproj; shares psum tags) ----
            qt_group(0, 0, False)
            qt_group(0, 1, True)

            kv_f32 = {}
            kv_q = {"k": [nc.sync, nc.scalar, nc.sync, nc.scalar],
                    "v": [nc.scalar, nc.gpsimd, nc.scalar, nc.gpsimd]}
            kv_wts = {}
            for name, wten in (("k", wk), ("v", wv)):
                wts = []
                for q in range(4):
                    wt = sa.tile([128, 4, C], BF16, tag="wkv", bufs=3)
                    kv_q[name][q].dma_start(wt, wten[:, ds(4 * q, 4), :])
                    wts.append(wt)
                kv_wts[name] = wts
            for name in ("k", "v"):
                kvs = sa.tile([T, C], F32, name=f"{name}f32")
                kv_ps = [ops.tile([T, NQ], F32, tag="sT", bufs=2, name=f"kvps{name}{nh}") for nh in range(2)]
                for q in range(4):
                    wt = kv_wts[name][q]
                    for j in range(4):
                        for nh in range(2):
                            nc.tensor.matmul(
                                kv_ps[nh],
                                ehsT_sb[:, 4 * q + j, :],
                                wt[:, j, ds(512 * nh, 512)],
                                start=(q == 0 and j == 0),
                                stop=(q == 3 and j == 3),
                            )
                for nh in range(2):
                    if nh == 0:
                        nc.vector.tensor_copy(kvs[:, ds(0, 512)], kv_ps[nh])
                    else:
                        nc.scalar.copy(kvs[:, ds(512, 512)], kv_ps[nh])
                kv_f32[name] = kvs

            qt_group(0, 2, False)
            qt_group(0, 3, True)

            # kT / vT via f32 PE transpose into o128-tag psum slices
            for i in range(C // 128):
                for src_sb, dst in ((kv_f32["k"], kT_sb), (kv_f32["v"], vT_sb)):
                    tp = ops.tile([128, NQ], F32, tag="o128", bufs=3)
                    nc.tensor.transpose(
                        tp[:, ds(0, T)], src_sb[:, ds(128 * i, 128)], identf[:T, :T]
                    )
                    if i % 2 == 0:
                        nc.vector.tensor_copy(dst[:, i, :], tp[:, ds(0, T)])
                    else:
                        nc.scalar.copy(dst[:, i, :], tp[:, ds(0, T)])

            qt_group(0, 4, False)
            qt_group(0, 5, True)

            # M_h = v_h @ Wo_h, fp8, packed at stacked row 77h
            for h in range(H):
                i, po = h // 2, (h % 2) * 64
                m_stg = sa.tile([T, C], FP8, tag="mstg", bufs=3)
                for nh in range(2):
                    m_ps = ops.tile([T, NQ], F32, tag="sT", bufs=2)
                    nc.tensor.matmul(
                        m_ps,
                        vT_sb[ds(po, 64), i, :],
                        wo_sb[ds(po, 64), i, ds(512 * nh, 512)],
                        start=True,
                        stop=True,
                    )
                    if nh == 0:
                        nc.vector.tensor_copy(m_stg[:, ds(0, 512)], m_ps)
                    else:
                        nc.scalar.copy(m_stg[:, ds(512, 512)], m_ps)
                for (ti, pb, s0, nr) in _pack_pieces(h):
                    nc.gpsimd.dma_start(m8[ds(pb, nr), ti, :], m_stg[ds(s0, nr), :])

            qt_group(0, 6, False)
            qt_group(0, 7, True)

            # ---- stage B: software-pipelined q chunks ----
            pack_q = [nc.gpsimd, nc.gpsimd, nc.gpsimd, nc.sync]

            def sm_head(ci, h):
                qT = st[ci]["qT"]
                zst = zstacks[ci % 2]
                z8 = st[ci]["z8"]
                i, po = h // 2, (h % 2) * 64
                sT_ps = ops.tile([T, NQ], F32, tag="sT", bufs=2)
                nc.tensor.matmul(
                    sT_ps,
                    kT_sb[ds(po, 64), i, :],
                    qT[ds(po, 64), i, :],
                    start=True,
                    stop=True,
                )
                nc.scalar.activation(z8[:, h, :], sT_ps, AF.Exp)
                for (ti, pb, s0, nr) in _pack_pieces(h):
                    eng = pack_q[st[ci]["pk"] % len(pack_q)]
                    st[ci]["pk"] += 1
                    eng.dma_start(zst[ds(pb, nr), ti, :], z8[ds(s0, nr), h, :])

            def d_chain(ci):
                mk = st[ci]["mk"]
                zst = zstacks[ci % 2]
                dX = dinvXs[ci % 2]
                d_ps = ops.tile([96, NQ], F32, tag="dps", bufs=1)
                for kp in range(NKT // 2):
                    nc.tensor.matmul(
                        d_ps,
                        bones_sb[:, ds(2 * kp, 2), :],
                        zst[:, ds(2 * kp, 2), :],
                        start=(kp == 0),
                        stop=(kp == NKT // 2 - 1),
                        perf_mode=DR,
                    )
                # D = S0 - (1-mA)*SA - (1-mB)*SB
                u = work.tile([H, NQ], F32, tag="u", bufs=1)
                nc.vector.tensor_mul(u, d_ps[ds(32, H), :], mk[:, 2, :])
                v = work.tile([H, NQ], F32, tag="v", bufs=1)
                nc.vector.tensor_mul(v, d_ps[ds(64, H), :], mk[:, 3, :])
                w = work.tile([H, NQ], F32, tag="w", bufs=1)
                nc.vector.tensor_sub(w, d_ps[ds(0, H), :], u)
                dd = work.tile([H, NQ], F32, tag="dd", bufs=1)
                nc.vector.tensor_sub(dd, w, v)
                dinv = work.tile([H, NQ], F32, tag="dinv", bufs=1)
                nc.vector.reciprocal_approx_fast(dinv, dd)
                nc.gpsimd.tensor_copy(dX[ds(0, H), :], dinv)
                dA = work.tile([H, NQ], BF16, tag="dA", bufs=1)
                nc.vector.tensor_mul(dA, dinv, mk[:, 0, :])
                dB = work.tile([H, NQ], BF16, tag="dB", bufs=1)
                nc.vector.tensor_mul(dB, dinv, mk[:, 1, :])
                nc.gpsimd.dma_start(dX[ds(32, H), :], dA)
                nc.scalar.dma_start(dX[ds(64, H), :], dB)

            def norm(ci):
                zst, pst, dX = zstacks[ci % 2], pstacks[ci % 2], dinvXs[ci % 2]
                for kt in range(NKT):
                    db_ps = ops.tile([128, NQ], F32, tag="db", bufs=2)
                    nc.tensor.matmul(
                        db_ps, selx_sb[:, kt, :], dX, start=True, stop=True
                    )
                    nc.vector.tensor_mul(pst[:, kt, :], zst[:, kt, :], db_ps)

            def av(ci):
                pst, rs = pstacks[ci % 2], st[ci]["rs"]
                for qj in range(NQ // 128):
                    o_sb = work.tile([128, 2, 512], BF16, tag="osb", bufs=3)
                    for nh in range(2):
                        o_ps = ops.tile([128, 512], F32, tag="o128", bufs=3)
                        for kp in range(NKT // 2):
                            nc.tensor.matmul(
                                o_ps,
                                pst[:, ds(2 * kp, 2), ds(128 * qj, 128)],
                                m8[:, ds(2 * kp, 2), ds(512 * nh, 512)],
                                start=(kp == 0),
                                stop=False,
                                perf_mode=DR,
                            )
                        # residual (+bo) accumulated via identity matmul
                        nc.tensor.matmul(
                            o_ps,
                            ident,
                            rs[:, qj, ds(512 * nh, 512)],
                            start=False,
                            stop=True,
                        )
                        if nh == 0:
                            nc.vector.tensor_copy(o_sb[:, nh, :], o_ps)
                        else:
                            nc.scalar.copy(o_sb[:, nh, :], o_ps)
                    nc.sync.dma_start(out[:, ci, qj, :], o_sb)

            for ci in range(NCHUNK):
                if ci + 1 < NCHUNK:
                    load(ci + 1)
                for h in range(H):
                    sm_head(ci, h)
                    if ci + 1 < NCHUNK and h % 2 == 1:
                        ij = h // 2
                        if ci > 0 or ij < 4:
                            qt_group(ci + 1, ij, ij % 2 == 0)
                d_chain(ci)
                if ci == 0:
                    for ij in range(4, 8):   # filler PE work over the d-chain
                        qt_group(1, ij, ij % 2 == 0)
                else:
                    av(ci - 1)               # hides the DVE d-chain latency
                norm(ci)
            av(NCHUNK - 1)

    nc.compile()
    return nc


_NC_CACHE = {}


def get_nc():
    if "nc" not in _NC_CACHE:
        _NC_CACHE["nc"] = build_nc()
    return _NC_CACHE["nc"]


def _bf(x):
    return np.asarray(x, dtype=ml_dtypes.bfloat16)


def _f8(x):
    return np.asarray(x, dtype=ml_dtypes.float8_e4m3)


def make_in_maps(inputs):
    hs = np.asarray(inputs["hidden_states"], dtype=np.float32)
    ehs = np.asarray(inputs["encoder_hidden_states"], dtype=np.float32)
    mask_A = np.asarray(inputs["mask_A"], dtype=np.float32)
    mask_B = np.asarray(inputs["mask_B"], dtype=np.float32)
    Wq = np.asarray(inputs["Wq"], dtype=np.float32)
    Wk = np.asarray(inputs["Wk"], dtype=np.float32)
    Wv = np.asarray(inputs["Wv"], dtype=np.float32)
    Wo = np.asarray(inputs["Wo"], dtype=np.float32)
    bo = np.asarray(inputs["bo"], dtype=np.float32)
    idxA = list(np.asarray(inputs["token_indices_A"]).astype(np.int64) % T)
    idxB = list(np.asarray(inputs["token_indices_B"]).astype(np.int64) % T)
    setA = [t for t in idxA if t not in idxB]
    setB = list(dict.fromkeys(idxB))

    scale = 1.0 / np.sqrt(D)
    wq8 = _f8(Wq * (scale * SCL_Q))            # [C, C]
    wq8 = np.ascontiguousarray(
        wq8.reshape(C // 128, 128, C).transpose(1, 0, 2)
    )                                          # [128, 8, C]
    wk_bf = _bf(Wk * (1.0 / SCL_Q)).reshape(CT // 128, 128, C).transpose(1, 0, 2)
    wv_bf = _bf(Wv).reshape(CT // 128, 128, C).transpose(1, 0, 2)
    wo_bf = _bf(Wo).reshape(C // 128, 128, C).transpose(1, 0, 2)
    wk_bf = np.ascontiguousarray(wk_bf)
    wv_bf = np.ascontiguousarray(wv_bf)
    wo_bf = np.ascontiguousarray(wo_bf)

    # blockonesX [128, NKT, 48] fp8 and SelX [48, NKT, 128] bf16
    bones = np.zeros((128, NKT, 96), np.float32)
    sel = np.zeros((96, NKT, 128), np.float32)
    for r in range(RT):
        kt, p = r // 128, r % 128
        h, t = r // T, r % T
        bones[p, kt, h] = 1.0                   # S0: all t
        if t in setA:
            bones[p, kt, 32 + h] = 1.0          # SA
            sel[32 + h, kt, p] = 1.0            # dinv*mA
        elif t in setB:
            bones[p, kt, 64 + h] = 1.0          # SB
            sel[64 + h, kt, p] = 1.0            # dinv*mB
        else:
            sel[h, kt, p] = 1.0                 # dinv
    bones8 = _f8(bones)
    sel_bf = _bf(sel)

    masks = np.zeros((H, 4, HW), np.float32)
    masks[:, 0] = mask_A[None, :]
    masks[:, 1] = mask_B[None, :]
    masks[:, 2] = (1.0 - mask_A)[None, :]
    masks[:, 3] = (1.0 - mask_B)[None, :]
    masks_bf = _bf(masks)

    ehsT_l = None
    in_maps = []
    for b in range(B):
        hsb = hs[b]
        hs_bf = _bf(hsb)
        hsT8 = _f8(hs_bf.astype(np.float32).T)             # [C, HW]
        hsT8 = np.ascontiguousarray(
            hsT8.reshape(C // 128, 128, HW).transpose(1, 0, 2)
        )                                                   # [128, 8, HW]
        residb = _bf(hsb + bo[None, :])                     # [HW, C]
        residb = np.ascontiguousarray(
            residb.reshape(NCHUNK, NQ // 128, 128, C).transpose(2, 0, 1, 3)
        )                                                   # [128, 8, 4, C]
        ehsT_l = _bf(ehs[b].T.reshape(CT // 128, 128, T).transpose(1, 0, 2))
        in_maps.append(
            {
                "hsT8": hsT8,
                "resid": residb,
                "ehsT": np.ascontiguousarray(ehsT_l),
                "wq8": wq8,
                "wk": wk_bf,
                "wv": wv_bf,
                "wo": wo_bf,
                "bones": bones8,
                "selx": sel_bf,
                "masks": masks_bf,
            }
        )
    return in_maps


def unpack_out(arr):
    # arr [128, NCHUNK, NQ//128, C] -> [HW, C]
    return np.ascontiguousarray(
        np.asarray(arr).transpose(1, 2, 0, 3).reshape(HW, C)
    )


def kernel(**inputs) -> np.ndarray:
    from concourse.bass_utils import run_bass_kernel_spmd

    nc = get_nc()
    in_maps = make_in_maps(inputs)
    res = run_bass_kernel_spmd(nc, in_maps, core_ids=list(range(B)))
    return np.stack([unpack_out(res.results[b]["out"]) for b in range(B)]).astype(
        np.float32
    )
